# revision 2
# baseline (speedup 1.0000x reference)
"""Trainium2 Bass kernel for nn_AttnMatching (v5 design).

Reference math (reassociated, identical):
    ctx[n, d] = sum_t value_w[t] * self_attn[n, t, d]    # [N, D]  (tiny)
    out[n, l] = sum_d ctx[n, d] * emb[l, d]              # [N, L]

Sharding: vocab axis L split across 8 cores (6250 cols each), attnw
(value_w + self_attn) replicated; no communication.

Design (each point trace-verified on HW):
  - emb streamed as fp8 e3m4 (host scales by 64, folds 1/64 into
    value_w): halves the dominant HBM traffic. PE takes mixed
    bf16(lhsT=ctxT) x fp8e3(rhs=emb). rel_norm ~1.2e-2 (gate 2e-2).
  - every DMA gets its own CONTIGUOUS dram tensor (slicing a wider
    row-major tensor makes strided HBM reads).
  - one HWDGE ring alone sustains only ~90-140 GB/s (an SDMA engine
    doesn't pipeline descriptors within one queue); all three rings
    (sync, scalar, gpsimd) carry loads concurrently. attnw rides first
    on both HWDGE rings (ring FIFO = full priority), gpsimd carries
    the tail emb chunks.
  - DMA completion sems lag the last data byte by ~1.7us (write
    receipt); chunk order puts the late-receipt chunks' matmuls last
    and the final store covers a single 512-col unit.
  - ctx on the PE: 16 matmuls lhsT=sa_n [100,128] x rhs=w [100,1] ->
    ps_ctx[:, n]; one DVE copy -> ctxT bf16. Dummy warmup matmuls keep
    the PE clock ramping (cold PE runs mains ~3x slower).
  - mains: 13 matmuls of 512 cols, PSUM 3-banded at partition 0/32/64
    (quadrant 3 is broken in HW), slots in exec order across 5 psum
    tensors -> no bank reuse, tensor never waits on copies.
  - copies: 5 wide [80, 512] DVE copies (one per psum tensor; garbage
    bands come along for free; narrow [16,512] copies run at 1/8
    engine width). All on DVE: an ACT copy on scalar makes the
    framework put a 1.3us ACT_TABLE_LOAD at the head of scalar's
    stream, delaying its DMA issues.
  - no kernel epilogue at all: the compiler's own teardown drains DMAs
    and resets every semaphore; re-execution verified safe. Stores
    carry a completion inc nobody waits on (walrus requires a sync
    update on every DMA).
  - per-DMA completion semaphores (a shared per-queue sem is racy: the
    16 SDMA engines complete independently, so a count can be reached
    while an earlier DMA still has engines in flight).
"""

import os

import numpy as np

L = 50000
D = 128
T = 100
N = 16
NCORES = 8
LSH = L // NCORES          # 6250
SCALE = 64.0               # emb pre-scale into e3m4 range

MMW = 512                  # main matmul width (this walrus caps moving at 512)
NJ = (LSH + MMW - 1) // MMW  # 13 main matmuls (12x512 + 106)
NB = (NJ + 2) // 3         # 5 psum tensors, 3 banded outputs each
AW = 1 + N * D             # attnw cols: [w | sa t-major]
A0 = 1 + 8 * D             # attnw half 0: w + n0..7 (sync)
A1 = AW - A0               # attnw half 1: n8..15 (scalar)

N_WARMUP = int(os.environ.get("K_W", "6"))
N_FILL1 = int(os.environ.get("K_F1", "2"))   # between ctx halves
N_FILL2 = int(os.environ.get("K_F2", "1"))   # between ctx and mains
WARMW = 256                 # warmup matmul width
NUM_DEVICES = int(os.environ.get("K_NUM_DEVICES", str(NCORES)))

# emb col chunks: (c0, c1, queue). sync/scalar behind the attnw
# halves; gpsimd (SWDGE, slow start but free ring) takes the tail.
ECH = [(0, 1024, "sy"), (1024, 2048, "sy"),
       (2048, 3072, "sc"), (3072, 4096, "sc"),
       (4096, 5120, "gp"), (5120, LSH, "gp")]

# main execution order = expected chunk READY order (data end + ~1.7us
# receipt): gpsimd chunks end early; per-ring later chunks later.
# (chunk_idx, j). The final position is a lone 512-col unit (j7) so
# only one matmul+copy+store trails the last-arriving chunk.
MAIN_ORDER = [
    (4, 8), (4, 9), (5, 10), (5, 11), (5, 12),
    (0, 0), (0, 1), (2, 4), (2, 5),
    (1, 2), (1, 3), (3, 6), (3, 7),
]

# out: 3 contiguous dram tensors: psum-blocks 0+1, 2+3, 4
OUT_SHAPES = [[80, 2 * MMW], [80, 2 * MMW], [N, MMW]]

_cache = {}


def _jw(j):
    return min(MMW, LSH - j * MMW)


def _build():
    import concourse.bacc as bacc
    import concourse.mybir as mybir

    f32 = mybir.dt.float32
    bf16 = mybir.dt.bfloat16
    f8 = mybir.dt.float8e3

    nc = bacc.Bacc(
        "TRN2",
        target_bir_lowering=False,
        debug=False,
        enable_asserts=True,
        num_devices=NUM_DEVICES,
    )

    embd = [nc.dram_tensor(f"emb{c}", [D, c1 - c0], f8, kind="ExternalInput").ap()
            for c, (c0, c1, _q) in enumerate(ECH)]
    aw0 = nc.dram_tensor("aw0", [T, A0], bf16, kind="ExternalInput").ap()
    aw1 = nc.dram_tensor("aw1", [T, A1], bf16, kind="ExternalInput").ap()
    outd = [nc.dram_tensor(f"out{i}", sh, bf16, kind="ExternalOutput").ap()
            for i, sh in enumerate(OUT_SHAPES)]

    emb_sb = nc.alloc_sbuf_tensor("emb_sb", [D, LSH], f8).ap()
    attnw_sb = nc.alloc_sbuf_tensor("attnw_sb", [T, AW], bf16).ap()
    wscr = nc.alloc_sbuf_tensor("wscr", [D, D + WARMW], bf16).ap()
    ctxT = nc.alloc_sbuf_tensor("ctxT", [D, N], bf16).ap()
    out_sb = nc.alloc_sbuf_tensor("out_sb", [80, NB * MMW], bf16).ap()

    ps_warm = nc.alloc_psum_tensor("ps_warm", [D, WARMW], f32).ap()
    ps_ctx = nc.alloc_psum_tensor("ps_ctx", [D, N], f32).ap()
    ps_m = [nc.alloc_psum_tensor(f"ps_m{b}", [D, MMW], f32).ap() for b in range(NB)]

    qa0 = nc.alloc_semaphore("qa0")
    qa1 = nc.alloc_semaphore("qa1")
    qe = [nc.alloc_semaphore(f"qe{i}") for i in range(6)]
    z = nc.alloc_semaphore("z")
    cxm = nc.alloc_semaphore("cxm")
    cxv = nc.alloc_semaphore("cxv")
    mm = nc.alloc_semaphore("mm")
    cpv = nc.alloc_semaphore("cpv")
    st = nc.alloc_semaphore("st")

    def warmup():
        nc.tensor.matmul(ps_warm[:, :], lhsT=wscr[:, :D],
                         rhs=wscr[:, D:], start=True, stop=True)

    def ctx_mm(n):
        return nc.tensor.matmul(ps_ctx[:, n: n + 1],
                                lhsT=attnw_sb[:, 1 + D * n: 1 + D * (n + 1)],
                                rhs=attnw_sb[:, 0:1], start=True, stop=True)

    # ---- DMA issues (ring FIFO: attnw first on HWDGE rings) ----
    nc.sync.dma_start(attnw_sb[:, :A0], aw0[:, :]).then_inc(qa0, 16)
    nc.scalar.dma_start(attnw_sb[:, A0:], aw1[:, :]).then_inc(qa1, 16)
    ENG = {"sy": nc.sync, "sc": nc.scalar, "gp": nc.gpsimd}
    for c, (c0, c1, q) in enumerate(ECH):
        ENG[q].dma_start(emb_sb[:, c0:c1], embd[c][:, :]).then_inc(qe[c], 16)

    # ---- vector: warmup scratch, ctx copy, all wide out copies ----
    nc.vector.memset(wscr[:, :], 0.0).then_inc(z, 1)
    nc.vector.wait_ge(cxm, 1)
    nc.vector.tensor_copy(ctxT[:, :], ps_ctx[:, :]).then_inc(cxv, 1)
    for b in range(NB):
        nc.vector.wait_ge(mm, min(3 * b + 3, NJ))
        rows = 80 if b < 4 else N
        nc.vector.tensor_copy(out_sb[:rows, MMW * b: MMW * (b + 1)],
                              ps_m[b][:rows, :]).then_inc(cpv, 1)

    # ---- tensor: warmups, ctx, mains ----
    nc.tensor.wait_ge(z, 1)
    for _ in range(N_WARMUP):
        warmup()
    nc.tensor.wait_ge(qa0, 16)
    for n in range(8):
        ctx_mm(n)
    for _ in range(N_FILL1):
        warmup()
    nc.tensor.wait_ge(qa1, 16)
    for n in range(8, N):
        m = ctx_mm(n)
    m.then_inc(cxm, 1)
    for _ in range(N_FILL2):
        warmup()
    nc.tensor.wait_ge(cxv, 1)
    prev = None
    for p, (c, j) in enumerate(MAIN_ORDER):
        if c != prev:
            nc.tensor.wait_ge(qe[c], 16)
            prev = c
        a, b = p % 3, p // 3
        nc.tensor.matmul(ps_m[b][32 * a: 32 * a + N, : _jw(j)],
                         lhsT=ctxT[:, :],
                         rhs=emb_sb[:, MMW * j: MMW * j + _jw(j)],
                         start=True, stop=True).then_inc(mm, 1)

    # ---- stores (no completion wait: the NEFF teardown drains) ----
    nc.scalar.wait_ge(cpv, 2)
    nc.scalar.dma_start(outd[0][:, :], out_sb[:, : 2 * MMW]).then_inc(st, 16)
    nc.sync.wait_ge(cpv, 4)
    nc.sync.dma_start(outd[1][:, :], out_sb[:, 2 * MMW: 4 * MMW]).then_inc(st, 16)
    nc.sync.wait_ge(cpv, 5)
    nc.sync.dma_start(outd[2][:, :], out_sb[:N, 4 * MMW:]).then_inc(st, 16)

    nc.compile()
    return nc


def _get_nc():
    if "nc" not in _cache:
        _cache["nc"] = _build()
    return _cache["nc"]


def _make_in_maps(self_attn, emb_table, value_w):
    import ml_dtypes

    bf = ml_dtypes.bfloat16
    f8 = ml_dtypes.float8_e3m4
    self_attn = np.asarray(self_attn, dtype=np.float32)
    value_w = np.asarray(value_w, dtype=np.float32)
    attnw = np.empty((T, AW), dtype=bf)
    attnw[:, 0] = (value_w[0] / SCALE).astype(bf)
    attnw[:, 1:] = self_attn.transpose(1, 0, 2).reshape(T, N * D).astype(bf)
    aw0 = np.ascontiguousarray(attnw[:, :A0])
    aw1 = np.ascontiguousarray(attnw[:, A0:])
    embT = np.asarray(emb_table, dtype=np.float32)[1: L + 1].T * SCALE  # [D, L]
    np.clip(embT, -15.0, 15.0, out=embT)
    embT8 = embT.astype(f8)
    in_maps = []
    for k in range(NCORES):
        m = {"aw0": aw0, "aw1": aw1}
        sh = embT8[:, k * LSH: (k + 1) * LSH]
        for c, (c0, c1, _q) in enumerate(ECH):
            m[f"emb{c}"] = np.ascontiguousarray(sh[:, c0:c1])
        in_maps.append(m)
    return in_maps


def _unshard(res_k):
    o = [np.asarray(res_k[f"out{i}"]) for i in range(3)]
    full = np.empty((N, LSH), dtype=np.float32)
    for p, (_c, j) in enumerate(MAIN_ORDER):
        a, b = p % 3, p // 3
        w = _jw(j)
        if b < 4:
            full[:, j * MMW: j * MMW + w] = o[b // 2][
                32 * a: 32 * a + N, MMW * (b % 2): MMW * (b % 2) + w]
        else:
            full[:, j * MMW: j * MMW + w] = o[2][:, :w]
    return full


def run(self_attn, emb_table, value_w, trace=False):
    from concourse.bass_utils import run_bass_kernel_spmd

    nc = _get_nc()
    in_maps = _make_in_maps(self_attn, emb_table, value_w)
    res = run_bass_kernel_spmd(nc, in_maps, list(range(NCORES)), trace=trace)
    full = np.ascontiguousarray(
        np.concatenate([_unshard(res.results[k]) for k in range(NCORES)], axis=1),
        dtype=np.float32,
    )
    return full, res


def kernel(self_attn, mat2, traj, emb_table, value_w):
    full, _ = run(self_attn, emb_table, value_w, trace=False)
    return full
